# revision 6
# baseline (speedup 1.0000x reference)
"""MAGNN aggregation kernel for one TRN2 chip (8 NeuronCores), Bass/Tile.

Contract: kernel(**inputs) takes the FULL inputs of reference.setup_inputs()
and returns the FULL [50000, 128] float32 output.

Strategy (SPMD across 8 cores; all structure identical per core, data differs):
  Every scatter_mean hop is edge-processed as:
    dma_gather of source rows (random side, SWDGE descriptors) +
    one-hot matmul segment-reduce into PSUM 128-slot windows (sorted side).
  Work is sharded by each hop's scatter-target node range, so each core
  produces complete output rows for its slice; intermediate node tables are
  AllGather-ed (7 small collectives). The three hops that share the e1
  src-sorted index stream (B1, E12, E13) are fused into a single gather from
  a row-concatenated table (1536B rows) to amortize descriptor generation,
  which is the hardware bottleneck (~8ns/descriptor on the Q7 cores).
  The final DxD linears + attention softmax run replicated per row-slice.
"""
import sys
sys.path.insert(0, "/opt/trn_rl_repo")
import numpy as np

from concourse import bacc, tile, mybir
from concourse.bass_utils import run_bass_kernel_spmd

TRACE = False          # test.py sets True to collect a profile
LAST_EXEC_NS = None

C = 8
D = 128
N0 = 50000
E, E12 = 800000, 400000

B_N = 25000 // C          # 3125
W_N = (B_N + 127) // 128  # 25
CAP_N = W_N * 128         # 3200
B_0 = N0 // C             # 6250
W_0 = (B_0 + 127) // 128  # 49
CAP_0 = W_0 * 128         # 6272
AGROWS = C * CAP_N        # 25600

GCHUNK = 4096
GCHUNK_BE = 1024

F32 = mybir.dt.float32
I16 = mybir.dt.int16


# ---------------------------------------------------------------- host prep

def _wrap16(idx):
    return np.ascontiguousarray(idx.astype(np.int16).reshape(-1, 16).T)


def _tile128(arr):
    return np.ascontiguousarray(arr.reshape(-1, 128).T)


class StageSpec:
    def __init__(self, name, scat, gath, n_out_space, wts=None, split50k=False):
        self.name = name
        nE = len(scat)
        base = n_out_space // C
        W = (base + 127) // 128
        self.W = W
        core = scat // base
        local = scat - core * base
        win = local // 128
        slot = local % 128

        ngroups = 2 if split50k else 1
        self.ngroups = ngroups
        grp = (gath >= 25000).astype(np.int64) if split50k else np.zeros(nE, np.int64)
        gidx = gath - grp * 25000 if split50k else gath

        order = np.lexsort((win, grp, core))
        core_s, grp_s, win_s = core[order], grp[order], win[order]
        gidx_s, slot_s = gidx[order], slot[order]
        wts_s = wts[order] if wts is not None else None

        counts = np.zeros((C, ngroups, W), np.int64)
        np.add.at(counts, (core_s, grp_s, win_s), 1)
        tw = np.maximum((counts.max(axis=0) + 127) // 128, 1)
        self.tw = tw
        self.Tg = tw.sum(axis=1)

        # global tile numbering: window-major, groups interleaved
        self.tile_base = np.zeros((ngroups, W), np.int64)
        self.gpos = np.zeros((ngroups, W), np.int64)
        t = 0
        for w in range(W):
            for g in range(ngroups):
                self.tile_base[g, w] = t
                t += tw[g, w]
        for g in range(ngroups):
            p = 0
            for w in range(W):
                self.gpos[g, w] = p
                p += tw[g, w]
        self.T_total = t

        self.per_core = []
        core_bounds = np.searchsorted(core_s, np.arange(C + 1))
        for c in range(C):
            lo, hi = core_bounds[c], core_bounds[c + 1]
            cidx, cslot = gidx_s[lo:hi], slot_s[lo:hi]
            cwt = wts_s[lo:hi] if wts_s is not None else None
            L = self.T_total * 128
            idx_full = np.zeros(L, np.int64)
            slot_full = np.full(L, -1.0, np.float32)
            wt_full = np.zeros(L, np.float32)
            pos = 0
            for g in range(ngroups):
                for w in range(W):
                    n = counts[c, g, w]
                    b = self.tile_base[g, w] * 128
                    idx_full[b:b + n] = cidx[pos:pos + n]
                    slot_full[b:b + n] = cslot[pos:pos + n]
                    if cwt is not None:
                        wt_full[b:b + n] = cwt[pos:pos + n]
                    pos += n
            assert pos == hi - lo
            self.per_core.append((idx_full, slot_full, wt_full))

    def gather_arrays(self, c, chunk):
        idx_full, slot_full, wt_full = self.per_core[c]
        out_idx = []
        for g in range(self.ngroups):
            a = np.zeros(self.Tg[g] * 128, np.int64)
            for w in range(self.W):
                b = self.tile_base[g, w]
                n = self.tw[g, w] * 128
                a[self.gpos[g, w] * 128: self.gpos[g, w] * 128 + n] = \
                    idx_full[b * 128: b * 128 + n]
            Lg = int(np.ceil(len(a) / chunk) * chunk)
            ap = np.zeros(Lg, np.int64)
            ap[:len(a)] = a
            out_idx.append(_wrap16(ap))
        return out_idx, _tile128(slot_full), _tile128(wt_full)


def _degrees(idx, n):
    return np.maximum(np.bincount(idx, minlength=n).astype(np.float32), 1.0)


def _slice_pad(x, c, base, cap):
    out = np.zeros((cap, x.shape[1]), x.dtype)
    out[:base] = x[c * base:(c + 1) * base]
    return out


def _vec_slice_pad(v, c, base, cap):
    out = np.ones(cap, np.float32)
    out[:base] = v[c * base:(c + 1) * base]
    return out


def _scale_layout(v, W):
    return np.ascontiguousarray(v.reshape(W, 128).T)


STAGE_NAMES = ("A1", "A2", "A3", "C12", "C13", "D12", "D13", "B2", "B3")


def prep(inp):
    f32 = np.float32
    g = {k: np.asarray(inp[k]) for k in inp}
    e = {k: g[k].astype(np.int64) for k in
         ("e1_src", "e1_dst", "e2_src", "e2_dst", "e3_src", "e3_dst",
          "e12_src", "e12_dst", "e13_src", "e13_dst")}
    w1, w2, w3 = (g[k].astype(f32) for k in ("w1", "w2", "w3"))

    def ag_row(node):
        c = node // B_N
        return c * CAP_N + (node - c * B_N)

    specs = {}
    specs["A1"] = StageSpec("A1", e["e1_dst"], e["e1_src"], 25000, wts=w1, split50k=True)
    specs["A2"] = StageSpec("A2", e["e2_dst"], e["e2_src"], 25000, wts=w2, split50k=True)
    specs["A3"] = StageSpec("A3", e["e3_dst"], e["e3_src"], 25000, wts=w3, split50k=True)
    specs["C12"] = StageSpec("C12", e["e12_dst"], ag_row(e["e12_src"]), 25000)
    specs["C13"] = StageSpec("C13", e["e13_dst"], ag_row(e["e13_src"]), 25000)
    specs["D12"] = StageSpec("D12", e["e12_src"], ag_row(e["e12_dst"]), 25000)
    specs["D13"] = StageSpec("D13", e["e13_src"], ag_row(e["e13_dst"]), 25000)
    specs["B2"] = StageSpec("B2", e["e2_src"], ag_row(e["e2_dst"]), 50000)
    specs["B3"] = StageSpec("B3", e["e3_src"], ag_row(e["e3_dst"]), 50000)
    specs["BE"] = StageSpec("BE", e["e1_src"], ag_row(e["e1_dst"]), 50000, wts=w1)

    deg1t = _degrees(e["e1_dst"], 25000)
    deg2t = _degrees(e["e2_dst"], 25000)
    deg3t = _degrees(e["e3_dst"], 25000)
    deg1s = _degrees(e["e1_src"], N0)
    deg2s = _degrees(e["e2_src"], N0)
    deg3s = _degrees(e["e3_src"], N0)
    deg12t = _degrees(e["e12_dst"], 25000)
    deg13t = _degrees(e["e13_dst"], 25000)
    deg12s = _degrees(e["e12_src"], 25000)
    deg13s = _degrees(e["e13_src"], 25000)

    x_node = g["x_node"].astype(f32)
    x1, x2, x3 = g["x1"].astype(f32), g["x2"].astype(f32), g["x3"].astype(f32)

    scale_t = {"A1": 0.5 / deg1t, "A2": 0.5 / deg2t, "A3": 0.5 / deg3t,
               "C12": 0.5 / deg12t, "C13": 0.5 / deg13t,
               "D12": 0.5 / deg12s, "D13": 0.5 / deg13s}
    add_t = {"A1": 0.5 * x1, "A2": 0.5 * x2, "A3": 0.5 * x3,
             "C12": 0.5 * x2, "C13": 0.5 * x3, "D12": 0.5 * x1, "D13": 0.5 * x1}

    in_maps = []
    for c in range(C):
        m = {"x_node": x_node}
        for nm in STAGE_NAMES:
            sp = specs[nm]
            idxs, slots, wts = sp.gather_arrays(c, GCHUNK)
            for gi, ia in enumerate(idxs):
                m[f"idx_{nm}_{gi}"] = ia
            m[f"slot_{nm}"] = slots
            if nm.startswith("A"):
                m[f"wt_{nm}"] = wts
        sp = specs["BE"]
        idxs, slots, wts = sp.gather_arrays(c, GCHUNK_BE)
        m["idx_BE_0"] = idxs[0]
        m["slot_BE"] = slots
        m["wt_BE"] = wts

        m["scale_A1"] = _scale_layout(_vec_slice_pad(0.5 / deg1t, c, B_N, CAP_N), W_N)
        m["scale_A2"] = _scale_layout(_vec_slice_pad(0.5 / deg2t, c, B_N, CAP_N), W_N)
        m["scale_A3"] = _scale_layout(_vec_slice_pad(0.5 / deg3t, c, B_N, CAP_N), W_N)
        m["scale_C12"] = _scale_layout(_vec_slice_pad(0.5 / deg12t, c, B_N, CAP_N), W_N)
        m["scale_C13"] = _scale_layout(_vec_slice_pad(0.5 / deg13t, c, B_N, CAP_N), W_N)
        m["scale_D12"] = _scale_layout(_vec_slice_pad(0.5 / deg12s, c, B_N, CAP_N), W_N)
        m["scale_D13"] = _scale_layout(_vec_slice_pad(0.5 / deg13s, c, B_N, CAP_N), W_N)
        for nm in ("A1", "A2", "A3", "C12", "C13", "D12", "D13"):
            m[f"add_{nm}"] = _slice_pad(add_t[nm], c, B_N, CAP_N)

        m["scale_o1"] = _scale_layout(1.0 / _vec_slice_pad(deg1s, c, B_0, CAP_0), W_0)
        m["scale_o2"] = _scale_layout(1.0 / _vec_slice_pad(deg2s, c, B_0, CAP_0), W_0)
        m["scale_o3"] = _scale_layout(1.0 / _vec_slice_pad(deg3s, c, B_0, CAP_0), W_0)
        for wn in ("W1", "W2", "W3", "W121", "W131"):
            m["T" + wn] = np.ascontiguousarray(g[wn].astype(f32).T)
        for bn in ("b1", "b2", "b3", "b121", "b131"):
            m[bn] = g[bn].astype(f32).reshape(128, 1)
        avT = np.ascontiguousarray(g["att_vec"].astype(f32).T)
        for p in range(5):
            mk = np.zeros((128, 5), f32)
            mk[:, p] = avT[:, p]
            m[f"att_m{p}"] = mk
        m["iota"] = np.broadcast_to(np.arange(128, dtype=f32), (128, 128)).copy()
        m["ident"] = np.eye(128, dtype=f32)
        in_maps.append(m)

    return specs, in_maps


# ------------------------------------------------------------- device build

def build_program(specs):
    nc = bacc.Bacc("TRN2", target_bir_lowering=False, debug=False, num_devices=C)

    din = {}

    def dparam(name, shape, dtype=F32):
        din[name] = nc.dram_tensor(name, list(shape), dtype, kind="ExternalInput")
        return din[name]

    dparam("x_node", (N0, D))
    for nm in STAGE_NAMES:
        sp = specs[nm]
        for g in range(sp.ngroups):
            Lg = int(np.ceil(sp.Tg[g] * 128 / GCHUNK) * GCHUNK)
            dparam(f"idx_{nm}_{g}", (16, Lg // 16), I16)
        dparam(f"slot_{nm}", (128, sp.T_total))
        if nm.startswith("A"):
            dparam(f"wt_{nm}", (128, sp.T_total))
    spBE = specs["BE"]
    LgBE = int(np.ceil(spBE.Tg[0] * 128 / GCHUNK_BE) * GCHUNK_BE)
    dparam("idx_BE_0", (16, LgBE // 16), I16)
    dparam("slot_BE", (128, spBE.T_total))
    dparam("wt_BE", (128, spBE.T_total))
    for nm in ("A1", "A2", "A3", "C12", "C13", "D12", "D13"):
        dparam(f"scale_{nm}", (128, W_N))
        dparam(f"add_{nm}", (CAP_N, D))
    for nm in ("o1", "o2", "o3"):
        dparam(f"scale_{nm}", (128, W_0))
    for wn in ("TW1", "TW2", "TW3", "TW121", "TW131"):
        dparam(wn, (128, 128))
    for bn in ("b1", "b2", "b3", "b121", "b131"):
        dparam(bn, (128, 1))
    for p in range(5):
        dparam(f"att_m{p}", (128, 5))
    dparam("iota", (128, 128))
    dparam("ident", (128, 128))

    out_d = nc.dram_tensor("out", [CAP_0, D], F32, kind="ExternalOutput")

    # internal DRAM
    ag_in = {nm: nc.dram_tensor(f"agin_{nm}", [CAP_N, D], F32)
             for nm in ("A1", "A2", "A3", "C12", "C13", "D12", "D13")}
    ag_out = {nm: nc.dram_tensor(f"agout_{nm}", [AGROWS, D], F32, addr_space="Shared")
              for nm in ("A1", "A2", "A3", "C12", "C13", "D12", "D13")}
    t3 = nc.dram_tensor("t3", [AGROWS, 3 * D], F32)
    sums_d = {nm: nc.dram_tensor(f"sums_{nm}", [CAP_0, D], F32)
              for nm in ("o1", "o2", "o3", "o12", "o13")}

    rg = [list(range(C))]

    with tile.TileContext(nc) as tc:
        with (
            tc.tile_pool(name="const", bufs=1) as cpool,
            tc.tile_pool(name="scales", bufs=1) as scpool,
            tc.tile_pool(name="idx", bufs=6) as idxp,
            tc.tile_pool(name="g128", bufs=4) as gp128,
            tc.tile_pool(name="gbe", bufs=3) as gpbe,
            tc.tile_pool(name="slw", bufs=4) as slwp,
            tc.tile_pool(name="oh", bufs=2) as ohp,
            tc.tile_pool(name="fl", bufs=4) as flp,
            tc.tile_pool(name="fin", bufs=3) as fin,
            tc.tile_pool(name="z5", bufs=2) as z5p,
            tc.tile_pool(name="psA", bufs=2, space="PSUM") as psA,
            tc.tile_pool(name="psB", bufs=2, space="PSUM") as psB,
            tc.tile_pool(name="psF", bufs=3, space="PSUM") as psF,
        ):
            def const(name, shape, dtype=F32):
                t = cpool.tile(list(shape), dtype, tag=name)
                nc.sync.dma_start(t[:], din[name][:])
                return t

            iota = const("iota", (128, 128))
            ident = const("ident", (128, 128))
            att_m = [const(f"att_m{p}", (128, 5)) for p in range(5)]
            WT = [const(n, (128, 128)) for n in ("TW1", "TW2", "TW3", "TW121", "TW131")]
            BV = [const(n, (128, 1)) for n in ("b1", "b2", "b3", "b121", "b131")]
            sc_net = {}
            for nm in ("A1", "A2", "A3", "C12", "C13", "D12", "D13"):
                sc_net[nm] = const(f"scale_{nm}", (128, W_N))
            sc_out = [const(f"scale_o{k}", (128, W_0)) for k in (1, 2, 3)]

            def emit_reduce(nm, sp, table_aps, elem, chunk, weighted, flush):
                """table_aps: per-group DRAM AP to gather from."""
                TPC = chunk // 128
                gpool = gp128 if elem == 128 else gpbe
                cur = [dict() for _ in range(sp.ngroups)]

                def get_chunk(g, ci):
                    if ci not in cur[g]:
                        # retire older chunks
                        for k in list(cur[g]):
                            if k < ci:
                                del cur[g][k]
                        it = idxp.tile([128, chunk // 16], I16, tag=f"idx{chunk}")
                        nc.sync.dma_start(
                            it[:],
                            din[f"idx_{nm}_{g}"][:, ci * (chunk // 16):(ci + 1) * (chunk // 16)]
                            .unsqueeze(0).to_broadcast((8, 16, chunk // 16)))
                        gt = gpool.tile([128, TPC, elem], F32, tag=f"g{elem}")
                        nc.gpsimd.dma_gather(gt[:], table_aps[g], it[:], chunk, chunk,
                                             elem, single_packet=False)
                        cur[g][ci] = gt
                    return cur[g][ci]

                for w in range(sp.W):
                    Kw = int(sp.tw[:, w].sum())
                    col0 = int(sp.tile_base[0, w])
                    sl = slwp.tile([128, Kw], F32, tag="slot")
                    nc.sync.dma_start(sl[:], din[f"slot_{nm}"][:, col0:col0 + Kw])
                    if weighted:
                        wt = slwp.tile([128, Kw], F32, tag="wt")
                        nc.sync.dma_start(wt[:], din[f"wt_{nm}"][:, col0:col0 + Kw])
                    oh = ohp.tile([128, Kw, 128], F32, tag="oh")
                    # unweighted one-hots in batches of 4
                    q = 0
                    while q < Kw:
                        r = min(4, Kw - q)
                        nc.vector.tensor_tensor(
                            oh[:, q:q + r, :],
                            sl[:, q:q + r].unsqueeze(-1).to_broadcast((128, r, 128)),
                            iota[:].unsqueeze(1).to_broadcast((128, r, 128)),
                            mybir.AluOpType.is_equal)
                        q += r
                    ohw = None
                    if weighted:
                        ohw = ohp.tile([128, Kw, 128], F32, tag="ohw")
                        for k in range(Kw):
                            nc.vector.tensor_scalar(
                                ohw[:, k, :], iota[:], sl[:, k:k + 1], wt[:, k:k + 1],
                                mybir.AluOpType.is_equal, mybir.AluOpType.mult)
                    # matmuls
                    if elem == 128:
                        ps = psA.tile([128, 128], F32, tag="ps")
                        lhs = ohw if weighted else oh
                        k = 0
                        for g in range(sp.ngroups):
                            for t in range(int(sp.tw[g, w])):
                                lt = int(sp.gpos[g, w]) + t
                                gt = get_chunk(g, lt // TPC)
                                kk = (int(sp.tile_base[g, w]) + t) - col0
                                nc.tensor.matmul(ps[:], lhs[:, kk, :],
                                                 gt[:, lt % TPC, :],
                                                 start=(k == 0), stop=(k == Kw - 1))
                                k += 1
                        flush(w, ps)
                    else:  # BE fused: unweighted cols 0:128, weighted cols 128:384
                        ps_u = psA.tile([128, 128], F32, tag="ps")
                        ps_w = psB.tile([128, 256], F32, tag="psw")
                        g = 0
                        for t in range(int(sp.tw[g, w])):
                            lt = int(sp.gpos[g, w]) + t
                            gt = get_chunk(g, lt // TPC)
                            kk = t
                            last = (t == int(sp.tw[g, w]) - 1)
                            nc.tensor.matmul(ps_u[:], oh[:, kk, :],
                                             gt[:, lt % TPC, 0:128],
                                             start=(t == 0), stop=last)
                            nc.tensor.matmul(ps_w[:], ohw[:, kk, :],
                                             gt[:, lt % TPC, 128:384],
                                             start=(t == 0), stop=last)
                        flush(w, (ps_u, ps_w))

            def flush_net(nm):
                def f(w, ps):
                    ad = flp.tile([128, 128], F32, tag="ad")
                    nc.sync.dma_start(ad[:], din[f"add_{nm}"][w * 128:(w + 1) * 128, :])
                    sb = flp.tile([128, 128], F32, tag="fl")
                    nc.vector.scalar_tensor_tensor(
                        sb[:], ps[:], sc_net[nm][:, w:w + 1], ad[:],
                        mybir.AluOpType.mult, mybir.AluOpType.add)
                    nc.sync.dma_start(ag_in[nm][w * 128:(w + 1) * 128, :], sb[:])
                return f

            def flush_raw(dst):
                def f(w, ps):
                    sb = flp.tile([128, 128], F32, tag="fl")
                    nc.scalar.copy(sb[:], ps[:])
                    nc.sync.dma_start(dst[w * 128:(w + 1) * 128, :], sb[:])
                return f

            def flush_be(w, pss):
                ps_u, ps_w = pss
                sb_u = flp.tile([128, 128], F32, tag="fl")
                nc.scalar.copy(sb_u[:], ps_u[:])
                nc.sync.dma_start(sums_d["o1"][w * 128:(w + 1) * 128, :], sb_u[:])
                sb_w = flp.tile([128, 256], F32, tag="flw")
                nc.scalar.copy(sb_w[:], ps_w[:])
                nc.sync.dma_start(sums_d["o12"][w * 128:(w + 1) * 128, :], sb_w[:, 0:128])
                nc.sync.dma_start(sums_d["o13"][w * 128:(w + 1) * 128, :], sb_w[:, 128:256])

            def emit_ag(nm):
                nc.gpsimd.collective_compute(
                    "AllGather", mybir.AluOpType.bypass, replica_groups=rg,
                    ins=[ag_in[nm][:]], outs=[ag_out[nm][:]])

            xn = din["x_node"]
            x_groups = [xn[0:25000, :], xn[25000:50000, :]]

            # A stages -> net tables -> AG
            for nm in ("A1", "A2", "A3"):
                emit_reduce(nm, specs[nm], x_groups, 128, GCHUNK, True, flush_net(nm))
                emit_ag(nm)
            # C stages (need net1)
            for nm in ("C12", "C13"):
                emit_reduce(nm, specs[nm], [ag_out["A1"][:]], 128, GCHUNK, False,
                            flush_net(nm))
                emit_ag(nm)
            # B2/B3 (need net2/net3) - emitted here so their gathers fill the
            # Pool queue while AGs for C/D are in flight
            emit_reduce("B2", specs["B2"], [ag_out["A2"][:]], 128, GCHUNK, False,
                        flush_raw(sums_d["o2"]))
            emit_reduce("B3", specs["B3"], [ag_out["A3"][:]], 128, GCHUNK, False,
                        flush_raw(sums_d["o3"]))
            # D stages (need n2 tables)
            for nm, src in (("D12", "C12"), ("D13", "C13")):
                emit_reduce(nm, specs[nm], [ag_out[src][:]], 128, GCHUNK, False,
                            flush_net(nm))
                emit_ag(nm)
            # build T3 = [net1 | n3_12 | n3_13]
            nc.sync.dma_start(t3[:, 0:128], ag_out["A1"][:])
            nc.sync.dma_start(t3[:, 128:256], ag_out["D12"][:])
            nc.sync.dma_start(t3[:, 256:384], ag_out["D13"][:])
            # BE fused stage
            emit_reduce("BE", specs["BE"], [t3[:]], 384, GCHUNK_BE, True, flush_be)

            # ---- final: per window of the N0 slice
            s_keys = ["o1", "o2", "o3", "o12", "o13"]
            s_scale = [0, 1, 2, 0, 0]   # scale_o index per path
            for w in range(W_0):
                zT = []
                for p in range(5):
                    ld = fin.tile([128, 128], F32, tag="ld")
                    nc.sync.dma_start(ld[:], sums_d[s_keys[p]][w * 128:(w + 1) * 128, :])
                    sc = fin.tile([128, 128], F32, tag="sc")
                    nc.scalar.mul(sc[:], ld[:], sc_out[s_scale[p]][:, w:w + 1])
                    pt = psF.tile([128, 128], F32, tag="pf")
                    nc.tensor.transpose(pt[:], sc[:], ident[:])
                    xT = fin.tile([128, 128], F32, tag="xT")
                    nc.scalar.copy(xT[:], pt[:])
                    zp = psF.tile([128, 128], F32, tag="pf")
                    nc.tensor.matmul(zp[:], WT[p][:], xT[:], start=True, stop=True)
                    z = z5p.tile([128, 128], F32, tag=f"z{p}")
                    nc.scalar.activation(z[:], zp[:],
                                         mybir.ActivationFunctionType.Relu,
                                         bias=BV[p][:, 0:1])
                    zT.append(z)
                sc_ps = psF.tile([128, 128], F32, tag="pf")
                for p in range(5):
                    nc.tensor.matmul(sc_ps[0:5, :], att_m[p][:], zT[p][:],
                                     start=(p == 0), stop=(p == 4))
                sc_sb = fin.tile([16, 128], F32, tag="s5")
                nc.scalar.copy(sc_sb[0:5, :], sc_ps[0:5, :])
                scT_ps = psF.tile([128, 8], F32, tag="pf")
                nc.tensor.transpose(scT_ps[:, 0:5], sc_sb[0:5, :], ident[0:5, 0:5])
                scT = fin.tile([128, 8], F32, tag="scT")
                nc.scalar.copy(scT[:, 0:5], scT_ps[:, 0:5])
                mx = fin.tile([128, 1], F32, tag="mx")
                nc.vector.reduce_max(mx[:], scT[:, 0:5], axis=mybir.AxisListType.X)
                nmx = fin.tile([128, 1], F32, tag="nmx")
                nc.vector.tensor_scalar_mul(nmx[:], mx[:], -1.0)
                ex = fin.tile([128, 8], F32, tag="ex")
                nc.scalar.activation(ex[:, 0:5], scT[:, 0:5],
                                     mybir.ActivationFunctionType.Exp,
                                     bias=nmx[:, 0:1])
                sm = fin.tile([128, 1], F32, tag="sm")
                nc.vector.reduce_sum(sm[:], ex[:, 0:5], axis=mybir.AxisListType.X)
                rc = fin.tile([128, 1], F32, tag="rc")
                nc.vector.reciprocal(rc[:], sm[:])
                att = fin.tile([128, 8], F32, tag="att")
                nc.vector.tensor_scalar(att[:, 0:5], ex[:, 0:5], rc[:, 0:1], None,
                                        mybir.AluOpType.mult)
                res = None
                for p in range(5):
                    zr_ps = psF.tile([128, 128], F32, tag="pf")
                    nc.tensor.transpose(zr_ps[:], zT[p][:], ident[:])
                    zr = fin.tile([128, 128], F32, tag="zr")
                    nc.scalar.copy(zr[:], zr_ps[:])
                    tmp = fin.tile([128, 128], F32, tag="tmp")
                    nc.vector.tensor_scalar(tmp[:], zr[:], att[:, p:p + 1], None,
                                            mybir.AluOpType.mult)
                    if res is None:
                        res = tmp
                    else:
                        res2 = fin.tile([128, 128], F32, tag="res")
                        nc.vector.tensor_tensor(res2[:], res[:], tmp[:],
                                                mybir.AluOpType.add)
                        res = res2
                nc.sync.dma_start(out_d[w * 128:(w + 1) * 128, :], res[:])

    nc.compile()
    return nc


# ------------------------------------------------------------------ runner

_CACHE = {}


def _edge_key(inp):
    h = 0
    for k in ("e1_src", "e1_dst", "e2_src", "e2_dst", "e3_src", "e3_dst",
              "e12_src", "e12_dst", "e13_src", "e13_dst"):
        a = np.asarray(inp[k])
        h ^= hash((k, a.shape, a.tobytes()))
    return h


def _install_trace_shim():
    import contextlib, ctypes, types
    so_path = "/opt/axon/libaxon_pjrt.so"
    try:
        lib = ctypes.CDLL(so_path)
        if not hasattr(lib, "axon_start_nrt_profile"):
            return False
        lib.axon_start_nrt_profile.argtypes = [ctypes.POINTER(ctypes.c_int64), ctypes.c_size_t]
        lib.axon_start_nrt_profile.restype = ctypes.c_int64
        lib.axon_stop_nrt_profile.argtypes = [ctypes.c_char_p]
        lib.axon_stop_nrt_profile.restype = ctypes.c_int64
    except OSError:
        return False

    @contextlib.contextmanager
    def hook(output_dir, device_ids):
        import jax
        jax.devices()
        if device_ids:
            ids = (ctypes.c_int64 * len(device_ids))(*device_ids)
            rc = lib.axon_start_nrt_profile(ids, len(device_ids))
        else:
            rc = lib.axon_start_nrt_profile(None, 0)
        if rc != 0:
            raise RuntimeError(f"axon_start_nrt_profile rc={rc}")
        try:
            yield
        finally:
            n = lib.axon_stop_nrt_profile(str(output_dir).encode())
            print(f"ntff profile: {n} file(s) -> {output_dir}", file=sys.stderr)

    mod = types.ModuleType("antenv.axon_hooks")
    mod.get_axon_ntff_profile_hook = lambda: hook
    mod.set_axon_ntff_profile_hook = lambda h: None
    sys.modules["antenv.axon_hooks"] = mod
    from concourse import bass_utils
    bass_utils.upload_artifacts = lambda tmpdir: tmpdir
    return True


def kernel(**inputs):
    global LAST_EXEC_NS
    key = _edge_key(inputs)
    specs, in_maps = prep(inputs)
    if key in _CACHE:
        nc = _CACHE[key]
    else:
        nc = build_program(specs)
        _CACHE.clear()
        _CACHE[key] = nc

    kwargs = {}
    if TRACE:
        if _install_trace_shim():
            kwargs["trace"] = True
    res = run_bass_kernel_spmd(nc, in_maps, list(range(C)), **kwargs)
    LAST_EXEC_NS = res.exec_time_ns
    out = np.empty((N0, D), np.float32)
    for c in range(C):
        out[c * B_0:(c + 1) * B_0] = res.results[c]["out"][:B_0]
    return out
